# revision 12
# baseline (speedup 1.0000x reference)
"""Trainium2 Bass kernel for nn_AttentionBlock (B=8, L=2048, E=512, FF=2048).

Strategy: data-parallel over batch - core b computes batch item b end-to-end
(no collectives). Activations live transposed ([feature, token], feature on
partitions) so every matmul contracts over the partition dim.

v2: the large matmuls run as fp8 DoubleRow (perf_mode=DoubleRow, K=256 per
instruction, 2x the f32r/fp16 FLOP rate measured on HW):
  - QKV projections: x8(e4m3) x w8(e4m3, x64 scale), evicted to q8/k8/v8
    (e4m3, x16 scale).
  - scoresT = k8^T q8 DR; exp on ACT with scale=SCALE/256 folding away the
    fp8 scales, output e5m2 (e4m3's 2^17 dynamic range cannot hold a
    global-bias softmax whose row maxima spread over e^12; e5m2's 2^32 can).
  - colsum via ones=16(e4m3) DR; AV = v8^T pexp DR (e5m2 moving x e4m3
    stationary); attn = AV * recip(colsum) (scales cancel).
  - FFN1: h8(e4m3) x w18(e4m3 x64) DR, evicted by ACT Relu(psum/64 + b1)
    into f16.
  - FFN2 stays fp16 (non-DR) - relu/w2 in fp8 would put the total error
    over the harness gate; fp16 keeps the error budget at ~1.3e-2.
The residual stream (xt), LN statistics (ones-matmul partition sums in
f32r), and all stats math stay f32/f32r as in v1. GPSIMD takes the
SBUF->SBUF elementwise work (squares for LN variance, h8 quantization)
so ACT (exp/relu/stats) and DVE (applies/residuals/reciprocals) keep up
with the doubled PE rate.

Error budget (host sim, exact RNE fp8 as verified on HW): ~1.33e-2 vs the
2e-2 gate.
"""
import math
from contextlib import ExitStack

import numpy as np
import ml_dtypes

import concourse.bass as bass
import concourse.bacc as bacc
import concourse.tile as tile
from concourse import mybir
from concourse.bass_utils import run_bass_kernel_spmd

P = 128
B, L, E, FF = 8, 2048, 512, 2048
NDOM = 32
EPS = 1e-5
SCALE = (1.0 / math.sqrt(E)) * 2.0 * math.log(NDOM)

EO = E // P           # 4  e-chunks
FO = FF // P          # 16 f-chunks
LC = 512              # l-chunk (matmul free dim)
NLC = L // LC         # 4  l-chunks
SB = L // P           # 16 s-blocks

SW = 64.0             # fp8 weight scale
SQ = 16.0             # q/k/v eviction scale
SMAX = 14.0           # assumed scores upper bound (measured 12.58 + margin)
EXPB = math.log(4096.0) - SMAX   # pexp peak ~1000, e5m2 max 57344

F32 = mybir.dt.float32
F32R = mybir.dt.float32r
F16 = mybir.dt.float16
F8 = mybir.dt.float8e4
F85 = mybir.dt.float8e5
AF = mybir.ActivationFunctionType
OP = mybir.AluOpType
DRM = mybir.MatmulPerfMode.DoubleRow
E4 = ml_dtypes.float8_e4m3

_TRACE = False
_DEBUG = False
LAST_RESULT = None
_CACHE = {}


def _round_fp32r(x):
    """Round-to-nearest-even fp32 -> fp32r (low 12 mantissa bits cleared)."""
    u = np.ascontiguousarray(x, dtype=np.float32).view(np.uint32)
    frac = u & np.uint32(0xFFF)
    base = u & np.uint32(0xFFFFF000)
    up = (frac > 0x800) | ((frac == 0x800) & (((u >> 12) & 1) == 1))
    return (base + np.where(up, np.uint32(0x1000), np.uint32(0))).view(np.float32)


def _build(ln1_trivial, ln2_trivial, b2_zero):
    nc = bacc.Bacc("TRN2", debug=False, target_bir_lowering=False, num_devices=B)

    xt_d = nc.dram_tensor("xt", [E, L], F32R, kind="ExternalInput")
    x8_d = nc.dram_tensor("x8", [E, L], F8, kind="ExternalInput")
    wq_d = nc.dram_tensor("wq8", [E, E], F8, kind="ExternalInput")
    wk_d = nc.dram_tensor("wk8", [E, E], F8, kind="ExternalInput")
    wv_d = nc.dram_tensor("wv8", [E, E], F8, kind="ExternalInput")
    w1_d = nc.dram_tensor("w18", [E, FF], F8, kind="ExternalInput")
    w2_d = nc.dram_tensor("w2f", [FF, E], F16, kind="ExternalInput")
    b1_d = nc.dram_tensor("b1v", [FF], F32, kind="ExternalInput")
    b2_d = None if b2_zero else nc.dram_tensor("b2v", [E], F32, kind="ExternalInput")
    ln1w_d = ln1b_d = ln2w_d = ln2b_d = None
    if not ln1_trivial:
        ln1w_d = nc.dram_tensor("ln1w", [E], F32, kind="ExternalInput")
        ln1b_d = nc.dram_tensor("ln1b", [E], F32, kind="ExternalInput")
    if not ln2_trivial:
        ln2w_d = nc.dram_tensor("ln2w", [E], F32, kind="ExternalInput")
        ln2b_d = nc.dram_tensor("ln2b", [E], F32, kind="ExternalInput")
    out_d = nc.dram_tensor("outt", [E, L], F32, kind="ExternalOutput")
    dbg = {}
    if _DEBUG:
        for nm, shp, dt in [("dy", [E, L], F32R), ("dh", [E, L], F32),
                            ("dh8", [E, L], F8), ("dk8", [E, L], F8),
                            ("dq8", [E, LC], F8), ("dv8", [L, E], F8),
                            ("dpe", [L, LC], mybir.dt.float8e5),
                            ("dr1", [FF, LC], F16), ("dy2", [E, L], F32R)]:
            dbg[nm] = nc.dram_tensor(nm, shp, dt, kind="ExternalOutput")

    xt_r = xt_d.ap().rearrange("(eo p) l -> p eo l", p=P)
    x8_r = x8_d.ap().rearrange("(eo p) l -> p eo l", p=P)
    wq_r = wq_d.ap().rearrange("(eo p) f -> p eo f", p=P)
    wk_r = wk_d.ap().rearrange("(eo p) f -> p eo f", p=P)
    wv_r = wv_d.ap().rearrange("(eo p) f -> p eo f", p=P)
    w1_r = w1_d.ap().rearrange("(eo p) f -> p eo f", p=P)
    w2_r = w2_d.ap().rearrange("(fo p) e -> p fo e", p=P)
    out_r = out_d.ap().rearrange("(eo p) l -> p eo l", p=P)

    with tile.TileContext(nc) as tc, ExitStack() as stk:
        const = stk.enter_context(tc.tile_pool(name="const", bufs=1))
        px = stk.enter_context(tc.tile_pool(name="px", bufs=1))
        pstat = stk.enter_context(tc.tile_pool(name="pstat", bufs=1))
        ph = stk.enter_context(tc.tile_pool(name="ph", bufs=2))
        ph8 = stk.enter_context(tc.tile_pool(name="ph8", bufs=2))
        pysq = stk.enter_context(tc.tile_pool(name="pysq", bufs=1))
        pw1 = stk.enter_context(tc.tile_pool(name="pw1", bufs=1))
        pw2 = stk.enter_context(tc.tile_pool(name="pw2", bufs=1))

        ones_f = const.tile([P, P], F32)
        ones_r = const.tile([P, P], F32R)
        ones8 = const.tile([P, 2, P], F8)
        eps_t = const.tile([P, 1], F32)
        expb_t = const.tile([P, 1], F32)
        b1_t = const.tile([P, FO], F32)
        nc.vector.memset(ones_f[:], 1.0)
        nc.vector.tensor_copy(ones_r[:], ones_f[:])
        nc.vector.memset(ones8[:], SQ)
        nc.vector.memset(eps_t[:], EPS)
        nc.vector.memset(expb_t[:], EXPB)
        b1_r = b1_d.ap().rearrange("(fo p) -> p fo", p=P)
        b2_t = None
        if b2_d is not None:
            b2_t = const.tile([P, EO], F32)
            nc.sync.dma_start(b2_t[:], b2_d.ap().rearrange("(eo p) -> p eo", p=P))
        ln1w_t = ln1b_t = ln2w_t = ln2b_t = None
        if ln1w_d is not None:
            ln1w_t = const.tile([P, EO], F32)
            ln1b_t = const.tile([P, EO], F32)
            nc.sync.dma_start(ln1w_t[:], ln1w_d.ap().rearrange("(eo p) -> p eo", p=P))
            nc.sync.dma_start(ln1b_t[:], ln1b_d.ap().rearrange("(eo p) -> p eo", p=P))
        if ln2w_d is not None:
            ln2w_t = const.tile([P, EO], F32)
            ln2b_t = const.tile([P, EO], F32)
            nc.sync.dma_start(ln2w_t[:], ln2w_d.ap().rearrange("(eo p) -> p eo", p=P))
            nc.sync.dma_start(ln2b_t[:], ln2b_d.ap().rearrange("(eo p) -> p eo", p=P))

        xt = px.tile([P, EO, L], F32R)          # x^T, becomes y = x + attn in B
        w18 = pw1.tile([P, EO, FF], F8)
        w2f = pw2.tile([P, FO, E], F16)
        state = {}

        def ln_stats_rest(i, tag, s_ps, s2_ps, y_sl):
            """negmean/meansq on ACT, var+rstd via sqrt + fast reciprocal."""
            negmean = pstat.tile([P, LC], F32, tag="nm", name=f"nm{tag}_{i}")
            msq = pstat.tile([P, LC], F32, tag="msq", name=f"msq{tag}_{i}")
            ex2 = pstat.tile([P, LC], F32, tag="ex2", name=f"ex2{tag}_{i}")
            rstd = pstat.tile([P, LC], F32, tag="rstd", name=f"rstd{tag}_{i}")
            nc.scalar.activation(negmean[:], s_ps[:], AF.Copy, scale=-1.0 / E)
            nc.scalar.activation(msq[:], s_ps[:], AF.Square, scale=1.0 / E)
            nc.vector.tensor_scalar_mul(ex2[:], s2_ps[:], 1.0 / E)
            nc.vector.tensor_tensor(ex2[:], ex2[:], msq[:], OP.subtract)
            nc.scalar.activation(ex2[:], ex2[:], AF.Sqrt, bias=eps_t[:])
            nc.vector.reciprocal_approx_fast(rstd[:], ex2[:])
            return y_sl, negmean, rstd

        def ln1_apply(i):
            """h (f32, for the residual) on DVE + h8 (fp8, FFN1 operand) on
            GPSIMD."""
            y_sl, negmean, rstd = state.pop(("ln1", i))
            hf = ph.tile([P, EO, LC], F32, tag="h", name=f"h{i}")
            h8 = ph8.tile([P, EO, LC], F8, tag="h8", name=f"h8_{i}")
            for ec in range(EO):
                t = pstat.tile([P, LC], F32, tag="lnapp", name=f"la1_{i}_{ec}")
                nc.vector.tensor_tensor(t[:], y_sl[ec].bitcast(F32),
                                        negmean[:], OP.add)
                if ln1_trivial:
                    nc.vector.tensor_tensor(hf[:, ec, :], t[:], rstd[:], OP.mult)
                else:
                    nc.vector.tensor_tensor(t[:], t[:], rstd[:], OP.mult)
                    nc.scalar.activation(hf[:, ec, :], t[:], AF.Identity,
                                         bias=ln1b_t[:, ec:ec + 1],
                                         scale=ln1w_t[:, ec:ec + 1])
                nc.gpsimd.tensor_copy(h8[:, ec, :], hf[:, ec, :])
            if _DEBUG:
                ls_ = i * LC
                nc.sync.dma_start(
                    dbg["dh"].ap().rearrange("(eo p) l -> p eo l", p=P)[:, :, ls_:ls_ + LC],
                    hf[:])
                nc.sync.dma_start(
                    dbg["dh8"].ap().rearrange("(eo p) l -> p eo l", p=P)[:, :, ls_:ls_ + LC],
                    h8[:])
            state[("h", i)] = (hf, h8)

        with tc.tile_pool(name="pkv", bufs=1) as pkv, \
             tc.tile_pool(name="pw", bufs=1) as pw, \
             tc.tile_pool(name="px8", bufs=1) as px8, \
             tc.tile_pool(name="psMM", bufs=2, space="PSUM") as psMM:
            wq8 = pw.tile([P, EO, E], F8)
            wk8 = pw.tile([P, EO, E], F8)
            wv8 = pw.tile([P, EO, E], F8)
            x8 = px8.tile([P, EO, L], F8)
            k8 = pkv.tile([P, EO, L], F8)      # k^T [e, s]
            q8 = pkv.tile([P, EO, L], F8)      # q^T [e, l]
            v8 = pkv.tile([P, SB, E], F8)      # v natural [s, e]

            # ---------------- phase A: k, v (and q0) projections ----------
            # first-need DMA order across queues
            nc.sync.dma_start(wk8[:], wk_r)
            nc.scalar.dma_start(x8[:, :, 0:L // 2], x8_r[:, :, 0:L // 2])
            nc.gpsimd.dma_start(x8[:, :, L // 2:], x8_r[:, :, L // 2:])
            nc.sync.dma_start(wv8[:], wv_r)
            nc.sync.dma_start(wq8[:], wq_r)
            for eo in range(EO):
                (nc.scalar if eo % 2 == 0 else nc.gpsimd).dma_start(
                    xt[:, eo, :], xt_r[:, eo, :])
            nc.gpsimd.dma_start(w18[:], w1_r)
            nc.gpsimd.dma_start(w2f[:], w2_r)
            nc.scalar.dma_start(b1_t[:], b1_r)

            def dr_proj(out_t, fb, w_t, lc, evict_eng):
                """one [f-block, l-chunk] projection: 2 DR matmuls + evict."""
                ls = lc * LC
                pp = psMM.tile([P, LC], F32, tag="mm", name=f"pj{fb}_{lc}")
                for kk in range(0, EO, 2):
                    nc.tensor.matmul(pp[:], w_t[:, kk:kk + 2, fb * P:(fb + 1) * P],
                                     x8[:, kk:kk + 2, ls:ls + LC],
                                     start=(kk == 0), stop=(kk == EO - 2),
                                     perf_mode=DRM)
                if evict_eng is nc.scalar:
                    nc.scalar.activation(out_t[:, fb, ls:ls + LC], pp[:],
                                         AF.Copy, scale=SQ / SW)
                else:
                    evict_eng.tensor_scalar_mul(out_t[:, fb, ls:ls + LC], pp[:],
                                                SQ / SW)

            def v_proj(lb, evict_eng):
                """one [s-block, e] v tile: 2 DR matmuls + evict."""
                pp = psMM.tile([P, E], F32, tag="mm", name=f"vj{lb}")
                for kk in range(0, EO, 2):
                    nc.tensor.matmul(pp[:], x8[:, kk:kk + 2, lb * P:(lb + 1) * P],
                                     wv8[:, kk:kk + 2, :],
                                     start=(kk == 0), stop=(kk == EO - 2),
                                     perf_mode=DRM)
                if evict_eng is nc.scalar:
                    nc.scalar.activation(v8[:, lb, :], pp[:], AF.Copy,
                                         scale=SQ / SW)
                else:
                    evict_eng.tensor_scalar_mul(v8[:, lb, :], pp[:], SQ / SW)

            # k for all l first (scores chunk 0 needs all of k), then q
            # (chunk-major), then v; evictions cycled ACT/DVE/GPS (GPS is
            # ~2x slower per tile, so it takes every 5th)
            engs = [nc.scalar, nc.vector]
            ei = 0
            for lc in range(NLC):
                for fb in range(EO):
                    dr_proj(k8, fb, wk8, lc, engs[ei % 2]); ei += 1
            for lc in range(NLC):
                for fb in range(EO):
                    dr_proj(q8, fb, wq8, lc, engs[ei % 2]); ei += 1
            for lb in range(SB):
                v_proj(lb, engs[ei % 2]); ei += 1
            if _DEBUG:
                nc.sync.dma_start(
                    dbg["dk8"].ap().rearrange("(eo p) l -> p eo l", p=P), k8[:])
                nc.sync.dma_start(
                    dbg["dv8"].ap().rearrange("(lb p) e -> p lb e", p=P), v8[:])

            # ---------------- phase B: attention per l-chunk --------------
            with (
                tc.tile_pool(name="pp", bufs=2) as pp_pool,
                tc.tile_pool(name="paon", bufs=1) as paon,
                tc.tile_pool(name="psAO", bufs=4, space="PSUM") as psAO,
                tc.tile_pool(name="psCS", bufs=1, space="PSUM") as psCS,
                tc.tile_pool(name="psSB", bufs=1, space="PSUM") as psSB,
            ):
                # chunk-0 LN1 pieces, injected into chunk 1's attention stream
                def ln1c0_sq():
                    y_sl = [xt[:, ec, 0:LC] for ec in range(EO)]
                    ysq = pysq.tile([P, EO, LC], F32R, tag="ysq", name="ysq1_0")
                    for ec in range(EO):
                        nc.gpsimd.tensor_tensor(ysq[:, ec, :],
                                                y_sl[ec].bitcast(F32),
                                                y_sl[ec].bitcast(F32), OP.mult)
                    state["c0"] = (y_sl, ysq)

                def ln1c0_sum1():
                    y_sl, ysq = state["c0"]
                    s_ps = psSB.tile([P, LC], F32, tag="sums", name="s1_0")
                    for ec in range(EO):
                        nc.tensor.matmul(s_ps[:], ones_r[:], y_sl[ec],
                                         start=(ec == 0), stop=(ec == EO - 1))
                    negmean = pstat.tile([P, LC], F32, tag="nm", name="nm1_0")
                    nc.scalar.activation(negmean[:], s_ps[:], AF.Copy,
                                         scale=-1.0 / E)
                    msq = pstat.tile([P, LC], F32, tag="msq", name="msq1_0")
                    nc.scalar.activation(msq[:], s_ps[:], AF.Square, scale=1.0 / E)
                    state["c0b"] = (negmean, msq)

                def ln1c0_sum2():
                    y_sl, ysq = state.pop("c0")
                    negmean, msq = state.pop("c0b")
                    s2_ps = psSB.tile([P, LC], F32, tag="sums", name="s2_0")
                    for ec in range(EO):
                        nc.tensor.matmul(s2_ps[:], ones_r[:], ysq[:, ec, :],
                                         start=(ec == 0), stop=(ec == EO - 1))
                    ex2 = pstat.tile([P, LC], F32, tag="ex2", name="ex21_0")
                    rstd = pstat.tile([P, LC], F32, tag="rstd", name="rstd1_0")
                    nc.vector.tensor_scalar_mul(ex2[:], s2_ps[:], 1.0 / E)
                    nc.vector.tensor_tensor(ex2[:], ex2[:], msq[:], OP.subtract)
                    nc.scalar.activation(ex2[:], ex2[:], AF.Sqrt, bias=eps_t[:])
                    nc.vector.reciprocal_approx_fast(rstd[:], ex2[:])
                    state[("ln1", 0)] = ([xt[:, ec, 0:LC] for ec in range(EO)],
                                         negmean, rstd)

                if _DEBUG:
                    nc.sync.dma_start(
                        dbg["dq8"].ap().rearrange("(eo p) l -> p eo l", p=P)[:, :, 0:LC],
                        q8[:, :, 0:LC])
                for lc in range(NLC):
                    ls = lc * LC
                    pexp = pp_pool.tile([P, SB, LC], F85, tag="pexp",
                                        name=f"pexp{lc}")
                    ao = [psAO.tile([P, LC], F32, tag="ao", name=f"ao{lc}_{e}")
                          for e in range(EO)]
                    cs = psCS.tile([P, LC], F32, tag="cs", name=f"cs{lc}")

                    inject = {}
                    if lc == 1:
                        inject = {2: ln1c0_sq, 3: ln1c0_sum1, 5: ln1c0_sum2,
                                  6: lambda: ln1_apply(0)}

                    def scores(sb, lc=lc, ls=ls):
                        sp = psMM.tile([P, LC], F32, tag="mm",
                                       name=f"sp{lc}_{sb}")
                        for kk in range(0, EO, 2):
                            nc.tensor.matmul(
                                sp[:], k8[:, kk:kk + 2, sb * P:(sb + 1) * P],
                                q8[:, kk:kk + 2, ls:ls + LC],
                                start=(kk == 0), stop=(kk == EO - 2),
                                perf_mode=DRM)
                        nc.scalar.activation(pexp[:, sb, :], sp[:], AF.Exp,
                                             bias=expb_t[:],
                                             scale=SCALE / (SQ * SQ))

                    def av(sbp, pexp=pexp, ao=ao, cs=cs):
                        st = (sbp == 0)
                        sp = (sbp == SB // 2 - 1)
                        nc.tensor.matmul(cs[:], ones8[:],
                                         pexp[:, 2 * sbp:2 * sbp + 2, :],
                                         start=st, stop=sp, perf_mode=DRM)
                        for eb in range(EO):
                            nc.tensor.matmul(
                                ao[eb][:],
                                v8[:, 2 * sbp:2 * sbp + 2, eb * P:(eb + 1) * P],
                                pexp[:, 2 * sbp:2 * sbp + 2, :],
                                start=st, stop=sp, perf_mode=DRM)

                    scores(0)
                    scores(1)
                    for sbp in range(SB // 2):
                        if sbp + 1 < SB // 2:
                            scores(2 * sbp + 2)
                            scores(2 * sbp + 3)
                        av(sbp)
                        if sbp in inject:
                            inject[sbp]()

                    rcs = pstat.tile([P, LC], F32, tag="rcs")
                    nc.vector.reciprocal_approx_fast(rcs[:], cs[:])
                    # y = x + ao * rcs   (in place into xt)
                    for ec in range(EO):
                        aon = paon.tile([P, LC], F32, tag="aon")
                        nc.vector.tensor_tensor(aon[:], ao[ec][:], rcs[:], OP.mult)
                        nc.vector.tensor_tensor(
                            xt[:, ec, ls:ls + LC],
                            xt[:, ec, ls:ls + LC].bitcast(F32), aon[:], OP.add)
                    if _DEBUG:
                        if lc == 0:
                            nc.sync.dma_start(
                                dbg["dpe"].ap().rearrange("(sb p) l -> p sb l", p=P),
                                pexp[:])
                        nc.sync.dma_start(
                            dbg["dy"].ap().rearrange("(eo p) l -> p eo l", p=P)[:, :, ls:ls + LC],
                            xt[:, :, ls:ls + LC])

        # ---------------- phase C: LN1, FFN, LN2 per l-chunk ----------------
        with (
            tc.tile_pool(name="py2", bufs=1) as py2,
            tc.tile_pool(name="prelu", bufs=2) as prelu,
            tc.tile_pool(name="pout", bufs=1) as pout,
            tc.tile_pool(name="psF1", bufs=2, space="PSUM") as psF1,
            tc.tile_pool(name="psF2", bufs=4, space="PSUM") as psF2,
            tc.tile_pool(name="psS", bufs=2, space="PSUM") as psS,
        ):
            def ln_pre(i):
                """squares + partition sums + rstd for chunk i's LN1."""
                ls = i * LC
                y_sl = [xt[:, ec, ls:ls + LC] for ec in range(EO)]
                ysq = pysq.tile([P, EO, LC], F32R, tag="ysq", name=f"ysq1_{i}")
                for ec in range(EO):
                    nc.gpsimd.tensor_tensor(ysq[:, ec, :], y_sl[ec].bitcast(F32),
                                            y_sl[ec].bitcast(F32), OP.mult)
                s_ps = psS.tile([P, LC], F32, tag="sums", name=f"s1_{i}")
                s2_ps = psS.tile([P, LC], F32, tag="sums", name=f"s2_{i}")
                for ec in range(EO):
                    nc.tensor.matmul(s_ps[:], ones_r[:], y_sl[ec],
                                     start=(ec == 0), stop=(ec == EO - 1))
                for ec in range(EO):
                    nc.tensor.matmul(s2_ps[:], ones_r[:], ysq[:, ec, :],
                                     start=(ec == 0), stop=(ec == EO - 1))
                state[("ln1", i)] = ln_stats_rest(i, "1", s_ps, s2_ps, y_sl)

            def relu_start(i):
                relu1 = prelu.tile([P, FO, LC], F16, tag="relu1",
                                   name=f"relu1_{i}")
                state[("ffn", i)] = (relu1, None)

            def ao2_start(i):
                relu1, _ = state[("ffn", i)]
                ao2 = [psF2.tile([P, LC], F32, tag="ao2", name=f"ao2_{i}_{e}")
                       for e in range(EO)]
                state[("ffn", i)] = (relu1, ao2)

            def ffn1(i, fo):
                relu1, _ = state[("ffn", i)]
                _, h8 = state[("h", i)]
                fp = psF1.tile([P, LC], F32, tag="f1", name=f"fp{i}_{fo}")
                for kk in range(0, EO, 2):
                    nc.tensor.matmul(fp[:], w18[:, kk:kk + 2, fo * P:(fo + 1) * P],
                                     h8[:, kk:kk + 2, :],
                                     start=(kk == 0), stop=(kk == EO - 2),
                                     perf_mode=DRM)
                nc.scalar.activation(relu1[:, fo, :], fp[:], AF.Relu,
                                     bias=b1_t[:, fo:fo + 1], scale=1.0 / SW)

            def ffn2(i, fo):
                relu1, ao2 = state[("ffn", i)]
                for eb in range(EO):
                    nc.tensor.matmul(
                        ao2[eb][:], w2f[:, fo, eb * P:(eb + 1) * P],
                        relu1[:, fo, :],
                        start=(fo == 0), stop=(fo == FO - 1))

            def resid2(i):
                """z = ffn + h (+b2), squares + partition sums, per-ec
                interleaved so the post-FFN critical path is short."""
                _, ao2 = state[("ffn", i)]
                hf, _ = state.pop(("h", i))
                if b2_t is not None:
                    for ec in range(EO):
                        nc.vector.tensor_tensor(
                            hf[:, ec, :], hf[:, ec, :],
                            b2_t[:, ec:ec + 1].to_broadcast((P, LC)), OP.add)
                y2 = py2.tile([P, EO, LC], F32R, tag="y2", name=f"y2_{i}")
                ysq = pysq.tile([P, EO, LC], F32R, tag="ysq", name=f"ysq2_{i}")
                s_ps = psS.tile([P, LC], F32, tag="sums", name=f"s3_{i}")
                s2_ps = psS.tile([P, LC], F32, tag="sums", name=f"s4_{i}")
                for ec in range(EO):
                    nc.vector.tensor_tensor(y2[:, ec, :], ao2[ec][:],
                                            hf[:, ec, :], OP.add)
                    nc.gpsimd.tensor_tensor(ysq[:, ec, :],
                                            y2[:, ec, :].bitcast(F32),
                                            y2[:, ec, :].bitcast(F32), OP.mult)
                    nc.tensor.matmul(s_ps[:], ones_r[:], y2[:, ec, :],
                                     start=(ec == 0), stop=(ec == EO - 1))
                    nc.tensor.matmul(s2_ps[:], ones_r[:], ysq[:, ec, :],
                                     start=(ec == 0), stop=(ec == EO - 1))
                if _DEBUG:
                    ls_ = i * LC
                    if i == 0:
                        relu1_, _ = state[("ffn", i)]
                        nc.sync.dma_start(
                            dbg["dr1"].ap().rearrange("(fo p) l -> p fo l", p=P),
                            relu1_[:])
                    nc.sync.dma_start(
                        dbg["dy2"].ap().rearrange("(eo p) l -> p eo l", p=P)[:, :, ls_:ls_ + LC],
                        y2[:])
                state.pop(("ffn", i))
                state[("y2", i)] = (y2, s_ps, s2_ps)

            def ln2_full(i):
                y2, s_ps, s2_ps = state.pop(("y2", i))
                y2_sl = [y2[:, ec, :] for ec in range(EO)]
                _, negmean, rstd = ln_stats_rest(i, "2", s_ps, s2_ps, y2_sl)
                ls = i * LC
                outt = pout.tile([P, EO, LC], F32, tag="out", name=f"out{i}")
                # subtracts only need negmean - they hide under the ACT sqrt
                # and DVE reciprocal that produce rstd
                ts = []
                for ec in range(EO):
                    t = pstat.tile([P, LC], F32, tag=f"lnapp{ec}",
                                   name=f"la2_{i}_{ec}")
                    nc.vector.tensor_tensor(t[:], y2_sl[ec].bitcast(F32),
                                            negmean[:], OP.add)
                    ts.append(t)
                for ec in range(EO):
                    t = ts[ec]
                    if ln2_trivial:
                        nc.vector.tensor_tensor(outt[:, ec, :], t[:], rstd[:],
                                                OP.mult)
                    else:
                        nc.vector.tensor_tensor(t[:], t[:], rstd[:], OP.mult)
                        nc.scalar.activation(outt[:, ec, :], t[:], AF.Identity,
                                             bias=ln2b_t[:, ec:ec + 1],
                                             scale=ln2w_t[:, ec:ec + 1])
                    nc.sync.dma_start(out_r[:, ec, ls:ls + LC], outt[:, ec, :])

            # ---- pipelined emission (h(0) already computed in phase B);
            # chunk i-1's residual/LN2 ride inside chunk i's FFN stream so
            # the PE never waits on the DVE/GPSIMD epilogue ----
            for i in range(NLC):
                relu_start(i)
                ffn1(i, 0)
                ffn1(i, 1)
                if i > 0:
                    resid2(i - 1)            # frees ao2 psum banks
                ao2_start(i)
                ffn2(i, 0)
                ffn1(i, 2)
                ffn2(i, 1)
                ffn1(i, 3)
                ffn2(i, 2)
                if i > 0:
                    ln2_full(i - 1)          # PE-free; hides under FFN MMs
                if i + 1 < NLC:
                    ln_pre(i + 1)            # next chunk's LN1 stats
                ffn1(i, 4)
                ffn2(i, 3)
                ffn1(i, 5)
                ffn2(i, 4)
                ffn1(i, 6)
                ffn2(i, 5)
                if i + 1 < NLC:
                    ln1_apply(i + 1)         # h(i+1) ready before FFN(i) ends
                for fo in range(7, FO):
                    ffn1(i, fo)
                    ffn2(i, fo - 1)
                ffn2(i, FO - 1)
            resid2(NLC - 1)
            ln2_full(NLC - 1)

    nc.compile()
    return nc


def kernel(x, in_proj_w, ln1_w, ln1_b, ln2_w, ln2_b, w1, b1, w2, b2):
    global LAST_RESULT
    x = np.asarray(x, dtype=np.float32)
    in_proj_w = np.asarray(in_proj_w, dtype=np.float32)
    w1 = np.asarray(w1, dtype=np.float32)
    w2 = np.asarray(w2, dtype=np.float32)
    b1 = np.asarray(b1, dtype=np.float32)
    b2 = np.asarray(b2, dtype=np.float32)
    ln1_w = np.asarray(ln1_w, dtype=np.float32)
    ln1_b = np.asarray(ln1_b, dtype=np.float32)
    ln2_w = np.asarray(ln2_w, dtype=np.float32)
    ln2_b = np.asarray(ln2_b, dtype=np.float32)

    ln1_trivial = bool(np.all(ln1_w == 1.0) and np.all(ln1_b == 0.0))
    ln2_trivial = bool(np.all(ln2_w == 1.0) and np.all(ln2_b == 0.0))
    b2_zero = bool(np.all(b2 == 0.0))

    key = (ln1_trivial, ln2_trivial, b2_zero)
    if key not in _CACHE:
        _CACHE[key] = _build(*key)
    nc = _CACHE[key]

    wq8 = np.ascontiguousarray((in_proj_w[:E].T * SW)).astype(E4)     # [E, E]
    wk8 = np.ascontiguousarray((in_proj_w[E:2 * E].T * SW)).astype(E4)
    wv8 = np.ascontiguousarray((in_proj_w[2 * E:].T * SW)).astype(E4)
    w18 = np.ascontiguousarray(w1.T * SW).astype(E4)                  # [E, FF]
    w2f = np.ascontiguousarray(w2.T).astype(np.float16)               # [FF, E]

    in_maps = []
    for bb in range(B):
        xtb = np.ascontiguousarray(x[bb].T)
        m = {
            "xt": _round_fp32r(xtb),                  # [E, L]
            "x8": xtb.astype(E4),
            "wq8": wq8, "wk8": wk8, "wv8": wv8,
            "w18": w18, "w2f": w2f, "b1v": b1,
        }
        if not b2_zero:
            m["b2v"] = b2
        if not ln1_trivial:
            m["ln1w"] = ln1_w
            m["ln1b"] = ln1_b
        if not ln2_trivial:
            m["ln2w"] = ln2_w
            m["ln2b"] = ln2_b
        in_maps.append(m)

    res = run_bass_kernel_spmd(nc, in_maps, list(range(B)), trace=_TRACE)
    LAST_RESULT = res
    out = np.stack([np.ascontiguousarray(res.results[bb]["outt"].T)
                    for bb in range(B)])
    return out.astype(np.float32)
